# revision 40
# baseline (speedup 1.0000x reference)
"""Trainium2 Bass kernel for nn_EnsembleTransitionModel.

Sharding: model-parallel (expert-parallel). M=8 ensemble members across 8
NeuronCores; each core runs one full MLP over the whole batch. Inputs are
replicated, per-model weights are sharded.

All four matmul layers run in fp8 (e4m3) DoubleRow perf mode: each matmul
instruction contracts K=256 (two 128-k-tiles packed as pairs in the free
dim of both operands) at 2 MACs/cell/cycle — 2x bf16 throughput. fp32
accumulation in PSUM.

Precision plan (measured rel-err ~2.2e-3 vs the 2e-2 gate):
  - W1, Wh are scaled x64 before fp8 quantization (raw weights ~N(0,0.02)
    sit in e4m3's subnormal range); the 1/64 rides the existing per-feature
    affine applied by the scalar-engine Relu activation out of PSUM.
  - W3 stays unscaled (error is the same either way) so the output stage is
    a single fp32 vector add of the residual stream, exactly as in bf16.
  - The residual z_last (+ b3 folded) rides a separate fp32 path so the
    dominant output term stays full precision.
  - DIN=1925 is zero-padded to 2048 so the 5 a_hist rows fold into the main
    L1 matmul (no separate host-computed rank-5 term).

Layouts: activations feature-major (x^T: [features, batch]); x is packed
chunk-major in DRAM ([128, chunk, kpair, 2, 512]) so each chunk's DMA is
one fully-contiguous 8KB-per-partition transfer.
"""

import os
import sys

import numpy as np

for _p in ("/opt/trn_rl_repo", "/root/.axon_site/_ro/trn_rl_repo"):
    if os.path.isdir(_p) and _p not in sys.path:
        sys.path.insert(0, _p)

M = 8
B = 16384
HIST = 5
L = 384
A = 1
HID = 512
NHL = 2
DIN = L * HIST + A * HIST  # 1925
EPS = 1e-5

WS = 64.0  # fp8 weight pre-scale for W1/Wh (compensated in the affine)

NCH = 512  # batch columns per chunk (= max fp32 moving dim = 1 PSUM bank)
DINP = 2048  # DIN zero-padded to 16 k-tiles
KP1 = DINP // 256  # 8 L1 k-pairs (DoubleRow consumes 2 k-tiles per matmul)
HT = HID // 128  # 4 hidden feature tiles
HP = HT // 2  # 2 hidden k-pairs
LT = L // 128  # 3 output feature tiles
ZROW0 = (HIST - 1) * L  # 1536: first row of z_last within x^T

# vecs columns: [b1 (4) | s0 (4) | c0 (4) | s1 (4) | c1 (4) | sL1 (1)]
COL_B1 = 0
COL_S = lambda l: 4 + 8 * l
COL_C = lambda l: 8 + 8 * l
COL_SL1 = 4 + 8 * NHL
NVEC = COL_SL1 + 1


def build_bass(batch=B, zero_bias=True):
    """zero_bias=True (true for this model instance: b1/bh/beta/rmean all
    zero) routes half the activations to the Vector engine as a one-pass
    relu(scale*psum) tensor_scalar, halving the end-of-layer activation
    barrier the PE waits on. With nonzero biases everything stays on the
    Scalar engine's general affine activation path."""
    import concourse.bacc as bacc
    import concourse.tile as tile
    from concourse import mybir

    f32 = mybir.dt.float32
    bf16 = mybir.dt.bfloat16
    f8 = mybir.dt.float8e4
    DR = mybir.MatmulPerfMode.DoubleRow
    Relu = mybir.ActivationFunctionType.Relu
    add = mybir.AluOpType.add
    mult = mybir.AluOpType.mult
    maxop = mybir.AluOpType.max

    nchunk = batch // NCH
    assert nchunk * NCH == batch

    nc = bacc.Bacc("TRN2", target_bir_lowering=False)
    xT = nc.declare_dram_parameter("xT", [128, nchunk, KP1, 2, NCH], f8, isOutput=False)
    zT = nc.declare_dram_parameter("zT", [128, nchunk, LT, NCH], bf16, isOutput=False)
    w1 = nc.declare_dram_parameter("w1", [128, KP1, 2, HID], f8, isOutput=False)
    wh = nc.declare_dram_parameter("wh", [128, NHL, HP, 2, HT, 128], f8, isOutput=False)
    w3 = nc.declare_dram_parameter("w3", [128, HP, 2, L], f8, isOutput=False)
    vecs = nc.declare_dram_parameter("vecs", [128, NVEC], f32, isOutput=False)
    outT = nc.declare_dram_parameter("outT", [L, batch], bf16, isOutput=True)

    with tile.TileContext(nc) as tc:
        with (
            tc.tile_pool(name="wt", bufs=1) as wpool,
            tc.tile_pool(name="x", bufs=4) as xpool,
            tc.tile_pool(name="z", bufs=2) as zpool,
            tc.tile_pool(name="h", bufs=2) as hpool,
            tc.tile_pool(name="o", bufs=3) as opool,
            tc.tile_pool(name="ps1", bufs=4, space="PSUM") as ps1pool,
            tc.tile_pool(name="psh", bufs=1, space="PSUM") as pshpool,
        ):
            # per-k-pair W1 tiles so the first matmul only waits on its own
            # 128KB slice, not the whole 1MB preload (DMAs are interleaved
            # with chunk-0 x slices, kp by kp)
            w1_sb = [
                wpool.tile([128, 2, HID], f8, tag=f"w1_{kp}", name=f"w1_{kp}")
                for kp in range(KP1)
            ]
            wh_sb = wpool.tile([128, NHL, HP, 2, HT, 128], f8, tag="wh")
            w3_sb = wpool.tile([128, HP, 2, L], f8, tag="w3")
            v_sb = wpool.tile([128, NVEC], f32, tag="vecs")

            # The PE never idles in steady state: the four L1 matmul chains
            # of chunk c+1 (1.7us of act-independent work each) are emitted
            # inside chunk c's hidden/out phases so every end-of-layer
            # activation barrier is covered by L1' work:
            #   [hl0 kp0|kp1] L1' ht0 [hl1 kp0|kp1] L1' ht1+ht2
            #   [out kp0|kp1] L1' ht3
            def act(out_sl, ps_sl, scol, bcol, eng, l1=False):
                # out = relu(scale*ps + bias); bias==0 on the fast V path
                if eng == "V" and zero_bias:
                    sc = 1.0 / WS if l1 else v_sb[:, scol : scol + 1]
                    nc.vector.tensor_scalar(out_sl, ps_sl, sc, 0.0, mult, maxop)
                else:
                    sc = COL_SL1 if l1 else scol
                    nc.scalar.activation(
                        out_sl,
                        ps_sl,
                        Relu,
                        bias=v_sb[:, bcol : bcol + 1],
                        scale=v_sb[:, sc : sc + 1],
                    )

            def l1_chain(ht, x_t, ps):
                for kp in range(KP1):
                    nc.tensor.matmul(
                        ps[:],
                        w1_sb[kp][:, :, ht * 128 : (ht + 1) * 128],
                        x_t[:, kp],
                        start=(kp == 0),
                        stop=(kp == KP1 - 1),
                        perf_mode=DR,
                    )

            # Chain-act engine assignment. ht2's act queues on V right after
            # the TT adds (both done ~1us before the boundary); ht3's two
            # half-acts split S (first half, overlaps chainB) / V (second
            # half, V idle by then, and DVE tensor_scalar at 256 cols is
            # ~80ns faster than the Scalar path) — the ht3B act is the
            # period-boundary gate for next chunk's hidden kp1 group.
            L1_ENG = ("V", "V", "V", ("S", "V"))

            def l1_piece(ht, x_t, h1n, colsplit=False):
                ps = ps1pool.tile([128, NCH], f32, tag="ps1", name=f"psl1_{ht}")
                dst = h1n[ht // 2]
                eng = L1_ENG[ht] if zero_bias else "S"
                if colsplit:
                    # ht3's act gates next chunk's hidden kp1 group at the
                    # period boundary. Run the chain as two 256-col
                    # half-chains into SEPARATE psum banks (same-bank PE-W +
                    # ACT-R would serialize): the first half's act overlaps
                    # the second half's matmuls, so h1'[1] is ready ~250ns
                    # after the chain instead of ~1us.
                    half = NCH // 2
                    for h_ in range(2):
                        sl = slice(h_ * half, (h_ + 1) * half)
                        psh_ = ps if h_ == 0 else ps1pool.tile(
                            [128, NCH], f32, tag="ps1", name=f"psl1_{ht}b"
                        )
                        for kp in range(KP1):
                            nc.tensor.matmul(
                                psh_[:, :half],
                                w1_sb[kp][:, :, ht * 128 : (ht + 1) * 128],
                                x_t[:, kp, :, sl],
                                start=(kp == 0),
                                stop=(kp == KP1 - 1),
                                perf_mode=DR,
                            )
                        act(dst[:, ht % 2, sl], psh_[:, :half], None,
                            COL_B1 + ht, eng[h_] if zero_bias else "S",
                            l1=True)
                else:
                    l1_chain(ht, x_t, ps)
                    act(dst[:, ht % 2, :], ps[:], None, COL_B1 + ht, eng, l1=True)

            def l1_begin(ht, x_t, n0):
                # open an L1' chain with its first n0 matmuls — used to slot
                # act-independent PE work between hl0's kp0 and kp1 groups
                # so the period-boundary act gate has two extra slots of slack
                ps = ps1pool.tile([128, NCH], f32, tag="ps1", name=f"psl1_{ht}")
                for kp in range(n0):
                    nc.tensor.matmul(
                        ps[:],
                        w1_sb[kp][:, :, ht * 128 : (ht + 1) * 128],
                        x_t[:, kp],
                        start=(kp == 0),
                        stop=False,
                        perf_mode=DR,
                    )
                return ps

            def l1_end(ht, x_t, h1n, ps, n0):
                for kp in range(n0, KP1):
                    nc.tensor.matmul(
                        ps[:],
                        w1_sb[kp][:, :, ht * 128 : (ht + 1) * 128],
                        x_t[:, kp],
                        start=False,
                        stop=(kp == KP1 - 1),
                        perf_mode=DR,
                    )
                act(h1n[ht // 2][:, ht % 2, :], ps[:], None, COL_B1 + ht,
                    L1_ENG[ht] if zero_bias else "S", l1=True)

            def hidden_kp0(l, hin):
                pss = [
                    pshpool.tile([128, NCH], f32, tag=f"ps2_{mt}", name=f"ps2_{mt}")
                    for mt in range(HT)
                ]
                for mt in range(HT):
                    nc.tensor.matmul(
                        pss[mt][:],
                        wh_sb[:, l, 0, :, mt, :],
                        hin[0][:],
                        start=True,
                        stop=False,
                        perf_mode=DR,
                    )
                return pss

            def hidden_kp1(l, hin, pss):
                hout = [
                    hpool.tile(
                        [128, 2, NCH], f8, tag=f"h{l + 2}_{p}", name=f"h{l + 2}_{p}"
                    )
                    for p in range(HP)
                ]
                for mt in range(HT):
                    nc.tensor.matmul(
                        pss[mt][:],
                        wh_sb[:, l, 1, :, mt, :],
                        hin[1][:],
                        start=False,
                        stop=True,
                        perf_mode=DR,
                    )
                    dst = hout[mt // 2]
                    # full-tile acts, alternating engines: each ~760-690ns
                    # and mt0/mt1 (the halves of hout[0]) land on different
                    # engines in parallel, so hout[0] is ready ~1.0us after
                    # its stop-MM — and it's half the instruction count of
                    # the S/V half-split (the ~360ns fixed cost per act
                    # instruction dominated the split's latency win)
                    eng = ("S" if mt % 2 == 0 else "V") if zero_bias else "S"
                    act(
                        dst[:, mt % 2, :],
                        pss[mt][:],
                        COL_S(l) + mt,
                        COL_C(l) + mt,
                        eng,
                    )
                return hout

            # ---- prologue: weights + x0 (sliced) + x1/x2, chunk-0 L1 ----
            # PE warm-up: the HAM clock gate holds the PE at 1.2GHz until it
            # has been busy ~3.4us. The first real matmul can't start before
            # the prologue DMAs land (~11us), so burn the wait on junk
            # matmuls over a memset tile — the real stream then opens at
            # 2.4GHz instead of paying ~1.7us of cold-clock penalty.
            warm_sb = wpool.tile([128, 2, NCH], f8, tag="warm")
            nc.vector.memset(warm_sb[:], 0.0)
            warm_ps = ps1pool.tile([128, NCH], f32, tag="ps1", name="warm_ps")
            NWARM = 8  # 8 cold MMs = ~3.4us = exactly one HAM window; ends
            # right as the first w1/x0 slices land (~11us)
            for i in range(NWARM):
                nc.tensor.matmul(
                    warm_ps[:],
                    warm_sb[:, :, :128],
                    warm_sb[:],
                    start=(i == 0),
                    stop=(i == NWARM - 1),
                    perf_mode=DR,
                )
            # split trigger issue across two queues: each DMA_DIRECT2D costs
            # ~600ns on its issuing queue, and 19 serial prologue triggers on
            # Sync alone (~12us) were pacing the whole warm-up region.
            # GpSimd is otherwise idle and can dispatch DMAs on trn2.
            x_c = xpool.tile([128, KP1, 2, NCH], f8, tag="x", name="x_0")
            x_n = None
            x_n2 = None
            if nchunk > 1:
                x_n = xpool.tile([128, KP1, 2, NCH], f8, tag="x", name="x_1")
            if nchunk > 2:
                x_n2 = xpool.tile([128, KP1, 2, NCH], f8, tag="x", name="x_2")
            for kp in range(KP1):
                nc.gpsimd.dma_start(out=w1_sb[kp][:], in_=w1[:, kp])
                nc.sync.dma_start(out=x_c[:, kp], in_=xT[:, 0, kp])
                # x1/x2 triggers early in the Sync queue: chunk 0's own L1'
                # chain (cover for chunk 1) reads x1 ~10us in and was
                # stalling ~4us behind the full x0 slice train
                if kp == 1 and x_n is not None:
                    nc.sync.dma_start(out=x_n[:], in_=xT[:, 1])
                if kp == 3 and x_n2 is not None:
                    nc.sync.dma_start(out=x_n2[:], in_=xT[:, 2])
            z_c = zpool.tile([128, LT, NCH], bf16, tag="z", name="z_0")
            nc.gpsimd.dma_start(out=z_c[:], in_=zT[:, 0])
            nc.gpsimd.dma_start(out=wh_sb[:], in_=wh[:])
            nc.gpsimd.dma_start(out=w3_sb[:], in_=w3[:])
            nc.gpsimd.dma_start(out=v_sb[:], in_=vecs[:])
            h1c = [
                hpool.tile([128, 2, NCH], f8, tag=f"h1_{p}", name=f"h1_{p}")
                for p in range(HP)
            ]
            # chunk-0 L1 kp-OUTER with all four accumulations open at once:
            # each kp step needs only its own 256KB (w1_kp + x0_kp) slice,
            # so the PE starts ~0.7us after the first DMA lands and stays
            # ~DMA-paced instead of waiting for the full 2MB preload
            # (ht-outer chains stalled ~11us here).
            pr_ps = [
                ps1pool.tile([128, NCH], f32, tag="ps1", name=f"ps0_{ht}")
                for ht in range(HT)
            ]
            for kp in range(KP1):
                for ht in range(HT):
                    nc.tensor.matmul(
                        pr_ps[ht][:],
                        w1_sb[kp][:, :, ht * 128 : (ht + 1) * 128],
                        x_c[:, kp],
                        start=(kp == 0),
                        stop=(kp == KP1 - 1),
                        perf_mode=DR,
                    )
            for ht in range(HT):
                act(
                    h1c[ht // 2][:, ht % 2, :],
                    pr_ps[ht][:],
                    None,
                    COL_B1 + ht,
                    "S" if ht % 2 == 0 else "V",
                    l1=True,
                )

            # The Tile scheduler's cost model rates DR matmuls ~2x faster
            # than hardware (106.7ns vs 216ns measured), so its static order
            # leaves the period boundary uncovered: next chunk's hidden-l0
            # MMs get placed right after the out MMs and stall ~600ns each on
            # the Vector TT adds freeing the shared ps2 banks. tile_wait_until
            # pins each phase to a monotone sim-time grid so the static order
            # is exactly: hl0 | l1p0 | hl1 | l1p1 | out | l1p2 | l1p3 — the
            # two trailing L1' chains (3.5us of act-independent PE work)
            # cover the TT drain (~2.1us) at every period boundary.
            PIN_MS = 0.002  # 2us sim-grid per phase
            NPH = 7

            def pin(c, k):
                return tc.tile_wait_until((NPH * c + k) * PIN_MS)

            for c in range(nchunk):
                last = c == nchunk - 1
                with pin(c, 0):
                    # prefetch x three chunks ahead: the chain filler MMs
                    # between hl0's kp0/kp1 groups read x(c+1) right at the
                    # period boundary, so it must be fully resident by then
                    if c + 3 < nchunk:
                        x_f = xpool.tile([128, KP1, 2, NCH], f8, tag="x", name="x_f")
                        nc.gpsimd.dma_start(out=x_f[:], in_=xT[:, c + 3])
                    z_n = None
                    h1n = None
                    if not last:
                        z_n = zpool.tile([128, LT, NCH], bf16, tag="z", name="z_n")
                        nc.gpsimd.dma_start(out=z_n[:], in_=zT[:, c + 1])
                        h1n = [
                            hpool.tile(
                                [128, 2, NCH], f8, tag=f"h1_{p}", name=f"h1n_{p}"
                            )
                            for p in range(HP)
                        ]

                    pss0 = hidden_kp0(0, h1c)
                if not last:
                    # two L1' chain MMs between hl0's kp0 and kp1 groups:
                    # kp1 reads h1[1] whose last act lands ~0.8us after the
                    # ht3 chain — these two slots turn a ~40ns race into a
                    # ~0.5us margin
                    with pin(c, 0.4):
                        l1p0_ps = l1_begin(0, x_n, 2)
                with pin(c, 0.7):
                    h2 = hidden_kp1(0, h1c, pss0)
                if not last:
                    with pin(c, 1):
                        l1_end(0, x_n, h1n, l1p0_ps, 2)
                with pin(c, 2):
                    pss1 = hidden_kp0(1, h2)
                    h3 = hidden_kp1(1, h2, pss1)
                if not last:
                    with pin(c, 3):
                        l1_piece(1, x_n, h1n)

                # ---- out: delta^T = W3^T h3; out = delta^T + (zlast+b3)^T
                # out psums recycle the hidden-layer banks (ps2_0..2), so the
                # ps1 ring stays dedicated to the four L1' chains. Per-lt
                # kp0/kp1 pairs (instead of all-kp0-then-all-kp1) let each
                # Vector TT start as soon as its own pair stops, so the TT
                # drain overlaps the remaining out MMs.
                with pin(c, 4):
                    pso = [
                        pshpool.tile([128, NCH], f32, tag=f"ps2_{lt}", name=f"pso_{lt}")
                        for lt in range(LT)
                    ]
                    for lt in range(LT):
                        nc.tensor.matmul(
                            pso[lt][:],
                            w3_sb[:, 0, :, lt * 128 : (lt + 1) * 128],
                            h3[0][:],
                            start=True,
                            stop=False,
                            perf_mode=DR,
                        )
                        nc.tensor.matmul(
                            pso[lt][:],
                            w3_sb[:, 1, :, lt * 128 : (lt + 1) * 128],
                            h3[1][:],
                            start=False,
                            stop=True,
                            perf_mode=DR,
                        )
                        ot = opool.tile([128, NCH], bf16, tag=f"o{lt}", name=f"o{lt}")
                        nc.vector.tensor_tensor(ot[:], pso[lt][:], z_c[:, lt, :], add)
                        nc.sync.dma_start(
                            out=outT[
                                lt * 128 : (lt + 1) * 128, c * NCH : (c + 1) * NCH
                            ],
                            in_=ot[:],
                        )
                if not last:
                    with pin(c, 5):
                        l1_piece(2, x_n, h1n)
                    with pin(c, 6):
                        l1_piece(3, x_n, h1n, colsplit=True)
                    h1c, z_c = h1n, z_n
                    x_n = x_n2
                    x_n2 = x_f if c + 3 < nchunk else None
    nc.compile()
    return nc


def _f8():
    import ml_dtypes

    return ml_dtypes.float8_e4m3


def prep_core_inputs(
    z_hist, a_hist, W1, b1, Wh, bh, gamma, beta, rmean, rvar, W3, b3
):
    """Host-side shard prep: returns per-model input dicts (xT shared)."""
    f8 = _f8()
    batch = z_hist.shape[0]
    nchunk = batch // NCH
    x = np.concatenate(
        [z_hist.reshape(batch, -1), a_hist.reshape(batch, -1)], axis=1
    ).astype(np.float32)
    xpad = np.zeros((batch, DINP), np.float32)
    xpad[:, :DIN] = x
    xq = xpad.astype(f8)  # quantize once, then pure byte shuffles
    xT8 = np.ascontiguousarray(
        xq.reshape(nchunk, NCH, DINP // 128, 128).transpose(3, 0, 2, 1)
    ).reshape(128, nchunk, KP1, 2, NCH)

    z_last = z_hist[:, -1, :].astype(np.float32)  # [batch, L]

    rstd = 1.0 / np.sqrt(rvar.astype(np.float64) + EPS)  # [NHL, M, HID]
    s_aff = (gamma * rstd).astype(np.float32)
    c_aff = ((bh - rmean) * gamma * rstd + beta).astype(np.float32)

    in_maps = []
    for m in range(M):
        w1p = np.zeros((DINP, HID), np.float32)
        w1p[:DIN] = W1[m] * WS
        w1h = np.ascontiguousarray(
            w1p.astype(f8).reshape(DINP // 128, 128, HID).transpose(1, 0, 2)
        ).reshape(128, KP1, 2, HID)

        whh = np.ascontiguousarray(
            (Wh[:, m] * WS)
            .astype(np.float32)
            .astype(f8)
            .reshape(NHL, HT, 128, HT, 128)
            .transpose(2, 0, 1, 3, 4)
        ).reshape(128, NHL, HP, 2, HT, 128)

        w3h = np.ascontiguousarray(
            W3[m].astype(np.float32).astype(f8).reshape(HT, 128, L).transpose(1, 0, 2)
        ).reshape(128, HP, 2, L)

        vecs = np.zeros((128, NVEC), np.float32)
        vecs[:, COL_B1 : COL_B1 + HT] = b1[m].reshape(HT, 128).T
        for l in range(NHL):
            vecs[:, COL_S(l) : COL_S(l) + HT] = (s_aff[l, m] / WS).reshape(HT, 128).T
            vecs[:, COL_C(l) : COL_C(l) + HT] = c_aff[l, m].reshape(HT, 128).T
        vecs[:, COL_SL1] = 1.0 / WS

        zb = z_last + b3[m][None, :]  # fold b3 into the residual stream
        # bf16 residual + bf16 output: halves the z-in and out DMA streams
        # (the DMA system was the steady-state pacing limit at 2.5MB/period);
        # adds ~2e-3 rel-err against a 2e-2 gate
        import ml_dtypes

        zTm = np.ascontiguousarray(
            zb.reshape(nchunk, NCH, LT, 128).transpose(3, 0, 2, 1)
        ).astype(ml_dtypes.bfloat16)  # [128, nchunk, LT, NCH]
        in_maps.append(
            {"xT": xT8, "zT": zTm, "w1": w1h, "wh": whh, "w3": w3h, "vecs": vecs}
        )
    return in_maps


def _reset_device():
    """Clear any exec-unit wedge a previous (profiled) session left behind."""
    try:
        import ctypes

        import jax

        jax.devices()
        lib = ctypes.CDLL("/opt/axon/libaxon_pjrt.so")
        if hasattr(lib, "axon_reset"):
            lib.axon_reset.restype = ctypes.c_int64
            lib.axon_reset()
    except Exception:
        pass


def is_zero_bias(inputs):
    """True iff every additive term of the per-layer affines is zero, i.e.
    the activations reduce to relu(scale * psum)."""
    return not (
        inputs["b1"].any()
        or inputs["bh"].any()
        or inputs["beta"].any()
        or inputs["rmean"].any()
    )


def kernel(**inputs):
    inputs = {k: np.asarray(v) for k, v in inputs.items()}
    in_maps = prep_core_inputs(**inputs)
    nc = build_bass(B, zero_bias=is_zero_bias(inputs))

    from concourse import bass_utils

    _reset_device()
    res = bass_utils.run_bass_kernel_spmd(nc, in_maps, core_ids=list(range(M)))
    out = np.stack(
        [np.ascontiguousarray(res.results[m]["outT"].T) for m in range(M)]
    )  # [M, B, L]
    return out.astype(np.float32)



# revision 42
# speedup vs baseline: 1.0248x; 1.0248x over previous
"""Trainium2 Bass kernel for nn_EnsembleTransitionModel.

Sharding: model-parallel (expert-parallel). M=8 ensemble members across 8
NeuronCores; each core runs one full MLP over the whole batch. Inputs are
replicated, per-model weights are sharded.

All four matmul layers run in fp8 (e4m3) DoubleRow perf mode: each matmul
instruction contracts K=256 (two 128-k-tiles packed as pairs in the free
dim of both operands) at 2 MACs/cell/cycle — 2x bf16 throughput. fp32
accumulation in PSUM.

Precision plan (measured rel-err ~2.2e-3 vs the 2e-2 gate):
  - W1, Wh are scaled x64 before fp8 quantization (raw weights ~N(0,0.02)
    sit in e4m3's subnormal range); the 1/64 rides the existing per-feature
    affine applied by the scalar-engine Relu activation out of PSUM.
  - W3 stays unscaled (error is the same either way) so the output stage is
    a single fp32 vector add of the residual stream, exactly as in bf16.
  - The residual z_last (+ b3 folded) rides a separate fp32 path so the
    dominant output term stays full precision.
  - DIN=1925 is zero-padded to 2048 so the 5 a_hist rows fold into the main
    L1 matmul (no separate host-computed rank-5 term).

Layouts: activations feature-major (x^T: [features, batch]); x is packed
chunk-major in DRAM ([128, chunk, kpair, 2, 512]) so each chunk's DMA is
one fully-contiguous 8KB-per-partition transfer.
"""

import os
import sys

import numpy as np

for _p in ("/opt/trn_rl_repo", "/root/.axon_site/_ro/trn_rl_repo"):
    if os.path.isdir(_p) and _p not in sys.path:
        sys.path.insert(0, _p)

M = 8
B = 16384
HIST = 5
L = 384
A = 1
HID = 512
NHL = 2
DIN = L * HIST + A * HIST  # 1925
EPS = 1e-5

WS = 64.0  # fp8 weight pre-scale for W1/Wh (compensated in the affine)

NCH = 512  # batch columns per chunk (= max fp32 moving dim = 1 PSUM bank)
DINP = 2048  # DIN zero-padded to 16 k-tiles
KP1 = DINP // 256  # 8 L1 k-pairs (DoubleRow consumes 2 k-tiles per matmul)
HT = HID // 128  # 4 hidden feature tiles
HP = HT // 2  # 2 hidden k-pairs
LT = L // 128  # 3 output feature tiles
ZROW0 = (HIST - 1) * L  # 1536: first row of z_last within x^T

# vecs columns: [b1 (4) | s0 (4) | c0 (4) | s1 (4) | c1 (4) | sL1 (1)]
COL_B1 = 0
COL_S = lambda l: 4 + 8 * l
COL_C = lambda l: 8 + 8 * l
COL_SL1 = 4 + 8 * NHL
NVEC = COL_SL1 + 1


def build_bass(batch=B, zero_bias=True):
    """zero_bias=True (true for this model instance: b1/bh/beta/rmean all
    zero) routes half the activations to the Vector engine as a one-pass
    relu(scale*psum) tensor_scalar, halving the end-of-layer activation
    barrier the PE waits on. With nonzero biases everything stays on the
    Scalar engine's general affine activation path."""
    import concourse.bacc as bacc
    import concourse.tile as tile
    from concourse import mybir

    f32 = mybir.dt.float32
    bf16 = mybir.dt.bfloat16
    f8 = mybir.dt.float8e4
    DR = mybir.MatmulPerfMode.DoubleRow
    Relu = mybir.ActivationFunctionType.Relu
    add = mybir.AluOpType.add
    mult = mybir.AluOpType.mult
    maxop = mybir.AluOpType.max

    nchunk = batch // NCH
    assert nchunk * NCH == batch

    nc = bacc.Bacc("TRN2", target_bir_lowering=False)
    xT = nc.declare_dram_parameter("xT", [128, nchunk, KP1, 2, NCH], f8, isOutput=False)
    zT = nc.declare_dram_parameter("zT", [128, nchunk, LT, NCH], bf16, isOutput=False)
    w1 = nc.declare_dram_parameter("w1", [128, KP1, 2, HID], f8, isOutput=False)
    wh = nc.declare_dram_parameter("wh", [128, NHL, HP, 2, HT, 128], f8, isOutput=False)
    w3 = nc.declare_dram_parameter("w3", [128, HP, 2, L], f8, isOutput=False)
    vecs = nc.declare_dram_parameter("vecs", [128, NVEC], f32, isOutput=False)
    outT = nc.declare_dram_parameter("outT", [L, batch], bf16, isOutput=True)

    with tile.TileContext(nc) as tc:
        with (
            tc.tile_pool(name="wt", bufs=1) as wpool,
            tc.tile_pool(name="x", bufs=4) as xpool,
            tc.tile_pool(name="z", bufs=2) as zpool,
            tc.tile_pool(name="h", bufs=2) as hpool,
            tc.tile_pool(name="o", bufs=3) as opool,
            tc.tile_pool(name="ps1", bufs=4, space="PSUM") as ps1pool,
            tc.tile_pool(name="psh", bufs=1, space="PSUM") as pshpool,
        ):
            # per-k-pair W1 tiles so the first matmul only waits on its own
            # 128KB slice, not the whole 1MB preload (DMAs are interleaved
            # with chunk-0 x slices, kp by kp)
            w1_sb = [
                wpool.tile([128, 2, HID], f8, tag=f"w1_{kp}", name=f"w1_{kp}")
                for kp in range(KP1)
            ]
            wh_sb = wpool.tile([128, NHL, HP, 2, HT, 128], f8, tag="wh")
            w3_sb = wpool.tile([128, HP, 2, L], f8, tag="w3")
            v_sb = wpool.tile([128, NVEC], f32, tag="vecs")

            # The PE never idles in steady state: the four L1 matmul chains
            # of chunk c+1 (1.7us of act-independent work each) are emitted
            # inside chunk c's hidden/out phases so every end-of-layer
            # activation barrier is covered by L1' work:
            #   [hl0 kp0|kp1] L1' ht0 [hl1 kp0|kp1] L1' ht1+ht2
            #   [out kp0|kp1] L1' ht3
            def act(out_sl, ps_sl, scol, bcol, eng, l1=False):
                # out = relu(scale*ps + bias); bias==0 on the fast V path
                if eng == "V" and zero_bias:
                    sc = 1.0 / WS if l1 else v_sb[:, scol : scol + 1]
                    nc.vector.tensor_scalar(out_sl, ps_sl, sc, 0.0, mult, maxop)
                else:
                    sc = COL_SL1 if l1 else scol
                    nc.scalar.activation(
                        out_sl,
                        ps_sl,
                        Relu,
                        bias=v_sb[:, bcol : bcol + 1],
                        scale=v_sb[:, sc : sc + 1],
                    )

            def l1_chain(ht, x_t, ps):
                for kp in range(KP1):
                    nc.tensor.matmul(
                        ps[:],
                        w1_sb[kp][:, :, ht * 128 : (ht + 1) * 128],
                        x_t[:, kp],
                        start=(kp == 0),
                        stop=(kp == KP1 - 1),
                        perf_mode=DR,
                    )

            # Chain-act engine assignment. ht2's act queues on V right after
            # the TT adds (both done ~1us before the boundary); ht3's two
            # half-acts split S (first half, overlaps chainB) / V (second
            # half, V idle by then, and DVE tensor_scalar at 256 cols is
            # ~80ns faster than the Scalar path) — the ht3B act is the
            # period-boundary gate for next chunk's hidden kp1 group.
            L1_ENG = ("V", "V", "V", ("S", "V"))

            def l1_piece(ht, x_t, h1n, colsplit=False):
                ps = ps1pool.tile([128, NCH], f32, tag="ps1", name=f"psl1_{ht}")
                dst = h1n[ht // 2]
                eng = L1_ENG[ht] if zero_bias else "S"
                if colsplit:
                    # ht3's act gates next chunk's hidden kp1 group at the
                    # period boundary. Run the chain as two 256-col
                    # half-chains into SEPARATE psum banks (same-bank PE-W +
                    # ACT-R would serialize): the first half's act overlaps
                    # the second half's matmuls, so h1'[1] is ready ~250ns
                    # after the chain instead of ~1us.
                    half = NCH // 2
                    for h_ in range(2):
                        sl = slice(h_ * half, (h_ + 1) * half)
                        psh_ = ps if h_ == 0 else ps1pool.tile(
                            [128, NCH], f32, tag="ps1", name=f"psl1_{ht}b"
                        )
                        for kp in range(KP1):
                            nc.tensor.matmul(
                                psh_[:, :half],
                                w1_sb[kp][:, :, ht * 128 : (ht + 1) * 128],
                                x_t[:, kp, :, sl],
                                start=(kp == 0),
                                stop=(kp == KP1 - 1),
                                perf_mode=DR,
                            )
                        act(dst[:, ht % 2, sl], psh_[:, :half], None,
                            COL_B1 + ht, eng[h_] if zero_bias else "S",
                            l1=True)
                else:
                    l1_chain(ht, x_t, ps)
                    act(dst[:, ht % 2, :], ps[:], None, COL_B1 + ht, eng, l1=True)

            def l1_begin(ht, x_t, n0):
                # open an L1' chain with its first n0 matmuls — used to slot
                # act-independent PE work between hl0's kp0 and kp1 groups
                # so the period-boundary act gate has two extra slots of slack
                ps = ps1pool.tile([128, NCH], f32, tag="ps1", name=f"psl1_{ht}")
                for kp in range(n0):
                    nc.tensor.matmul(
                        ps[:],
                        w1_sb[kp][:, :, ht * 128 : (ht + 1) * 128],
                        x_t[:, kp],
                        start=(kp == 0),
                        stop=False,
                        perf_mode=DR,
                    )
                return ps

            def l1_end(ht, x_t, h1n, ps, n0):
                for kp in range(n0, KP1):
                    nc.tensor.matmul(
                        ps[:],
                        w1_sb[kp][:, :, ht * 128 : (ht + 1) * 128],
                        x_t[:, kp],
                        start=False,
                        stop=(kp == KP1 - 1),
                        perf_mode=DR,
                    )
                act(h1n[ht // 2][:, ht % 2, :], ps[:], None, COL_B1 + ht,
                    L1_ENG[ht] if zero_bias else "S", l1=True)

            def hidden_kp0(l, hin):
                pss = [
                    pshpool.tile([128, NCH], f32, tag=f"ps2_{mt}", name=f"ps2_{mt}")
                    for mt in range(HT)
                ]
                for mt in range(HT):
                    nc.tensor.matmul(
                        pss[mt][:],
                        wh_sb[:, l, 0, :, mt, :],
                        hin[0][:],
                        start=True,
                        stop=False,
                        perf_mode=DR,
                    )
                return pss

            def hidden_kp1(l, hin, pss):
                hout = [
                    hpool.tile(
                        [128, 2, NCH], f8, tag=f"h{l + 2}_{p}", name=f"h{l + 2}_{p}"
                    )
                    for p in range(HP)
                ]
                for mt in range(HT):
                    nc.tensor.matmul(
                        pss[mt][:],
                        wh_sb[:, l, 1, :, mt, :],
                        hin[1][:],
                        start=False,
                        stop=True,
                        perf_mode=DR,
                    )
                    dst = hout[mt // 2]
                    # full-tile acts, alternating engines: each ~760-690ns
                    # and mt0/mt1 (the halves of hout[0]) land on different
                    # engines in parallel, so hout[0] is ready ~1.0us after
                    # its stop-MM — and it's half the instruction count of
                    # the S/V half-split (the ~360ns fixed cost per act
                    # instruction dominated the split's latency win)
                    eng = ("S" if mt % 2 == 0 else "V") if zero_bias else "S"
                    act(
                        dst[:, mt % 2, :],
                        pss[mt][:],
                        COL_S(l) + mt,
                        COL_C(l) + mt,
                        eng,
                    )
                return hout

            # ---- prologue: weights + x0 (sliced) + x1/x2, chunk-0 L1 ----
            # PE warm-up: the HAM clock gate holds the PE at 1.2GHz until it
            # has been busy ~3.4us. The first real matmul can't start before
            # the prologue DMAs land (~11us), so burn the wait on junk
            # matmuls over a memset tile — the real stream then opens at
            # 2.4GHz instead of paying ~1.7us of cold-clock penalty.
            warm_sb = wpool.tile([128, 2, NCH], f8, tag="warm")
            nc.vector.memset(warm_sb[:], 0.0)
            warm_ps = ps1pool.tile([128, NCH], f32, tag="ps1", name="warm_ps")
            NWARM = 8  # 8 cold MMs = ~3.4us = exactly one HAM window; ends
            # right as the first w1/x0 slices land (~11us)
            for i in range(NWARM):
                nc.tensor.matmul(
                    warm_ps[:],
                    warm_sb[:, :, :128],
                    warm_sb[:],
                    start=(i == 0),
                    stop=(i == NWARM - 1),
                    perf_mode=DR,
                )
            # split trigger issue across two queues: each DMA_DIRECT2D costs
            # ~600ns on its issuing queue, and 19 serial prologue triggers on
            # Sync alone (~12us) were pacing the whole warm-up region.
            # GpSimd is otherwise idle and can dispatch DMAs on trn2.
            x_c = xpool.tile([128, KP1, 2, NCH], f8, tag="x", name="x_0")
            x_n = None
            x_n2 = None
            if nchunk > 1:
                x_n = xpool.tile([128, KP1, 2, NCH], f8, tag="x", name="x_1")
            if nchunk > 2:
                x_n2 = xpool.tile([128, KP1, 2, NCH], f8, tag="x", name="x_2")
            for kp in range(KP1):
                nc.gpsimd.dma_start(out=w1_sb[kp][:], in_=w1[:, kp])
                nc.sync.dma_start(out=x_c[:, kp], in_=xT[:, 0, kp])
            if x_n is not None:
                nc.sync.dma_start(out=x_n[:], in_=xT[:, 1])
            if x_n2 is not None:
                nc.sync.dma_start(out=x_n2[:], in_=xT[:, 2])
            z_c = zpool.tile([128, LT, NCH], bf16, tag="z", name="z_0")
            nc.gpsimd.dma_start(out=z_c[:], in_=zT[:, 0])
            nc.gpsimd.dma_start(out=wh_sb[:], in_=wh[:])
            nc.gpsimd.dma_start(out=w3_sb[:], in_=w3[:])
            nc.gpsimd.dma_start(out=v_sb[:], in_=vecs[:])
            h1c = [
                hpool.tile([128, 2, NCH], f8, tag=f"h1_{p}", name=f"h1_{p}")
                for p in range(HP)
            ]
            # chunk-0 L1 kp-OUTER with all four accumulations open at once:
            # each kp step needs only its own 256KB (w1_kp + x0_kp) slice,
            # so the PE starts ~0.7us after the first DMA lands and stays
            # ~DMA-paced instead of waiting for the full 2MB preload
            # (ht-outer chains stalled ~11us here).
            pr_ps = [
                ps1pool.tile([128, NCH], f32, tag="ps1", name=f"ps0_{ht}")
                for ht in range(HT)
            ]
            for kp in range(KP1):
                for ht in range(HT):
                    nc.tensor.matmul(
                        pr_ps[ht][:],
                        w1_sb[kp][:, :, ht * 128 : (ht + 1) * 128],
                        x_c[:, kp],
                        start=(kp == 0),
                        stop=(kp == KP1 - 1),
                        perf_mode=DR,
                    )
            for ht in range(HT):
                act(
                    h1c[ht // 2][:, ht % 2, :],
                    pr_ps[ht][:],
                    None,
                    COL_B1 + ht,
                    "S" if ht % 2 == 0 else "V",
                    l1=True,
                )

            # The Tile scheduler's cost model rates DR matmuls ~2x faster
            # than hardware (106.7ns vs 216ns measured), so its static order
            # leaves the period boundary uncovered: next chunk's hidden-l0
            # MMs get placed right after the out MMs and stall ~600ns each on
            # the Vector TT adds freeing the shared ps2 banks. tile_wait_until
            # pins each phase to a monotone sim-time grid so the static order
            # is exactly: hl0 | l1p0 | hl1 | l1p1 | out | l1p2 | l1p3 — the
            # two trailing L1' chains (3.5us of act-independent PE work)
            # cover the TT drain (~2.1us) at every period boundary.
            PIN_MS = 0.002  # 2us sim-grid per phase
            NPH = 7

            def pin(c, k):
                return tc.tile_wait_until((NPH * c + k) * PIN_MS)

            for c in range(nchunk):
                last = c == nchunk - 1
                with pin(c, 0):
                    # prefetch x three chunks ahead: the chain filler MMs
                    # between hl0's kp0/kp1 groups read x(c+1) right at the
                    # period boundary, so it must be fully resident by then
                    if c + 3 < nchunk:
                        x_f = xpool.tile([128, KP1, 2, NCH], f8, tag="x", name="x_f")
                        nc.gpsimd.dma_start(out=x_f[:], in_=xT[:, c + 3])
                    z_n = None
                    h1n = None
                    if not last:
                        z_n = zpool.tile([128, LT, NCH], bf16, tag="z", name="z_n")
                        nc.gpsimd.dma_start(out=z_n[:], in_=zT[:, c + 1])
                        h1n = [
                            hpool.tile(
                                [128, 2, NCH], f8, tag=f"h1_{p}", name=f"h1n_{p}"
                            )
                            for p in range(HP)
                        ]

                    pss0 = hidden_kp0(0, h1c)
                # two L1' chain MMs between hl0's kp0 and kp1 groups:
                # kp1 reads h1[1] whose last act lands ~0.8us after the
                # ht3 chain — these two slots turn a ~40ns race into a
                # ~0.5us margin. Chunk 0 skips the filler: its h1 comes
                # from the prologue (no late-act gate) and x1 is still in
                # flight at that point.
                nfill = 2 if c > 0 else 0
                if not last and nfill:
                    with pin(c, 0.4):
                        l1p0_ps = l1_begin(0, x_n, nfill)
                with pin(c, 0.7):
                    h2 = hidden_kp1(0, h1c, pss0)
                if not last:
                    with pin(c, 1):
                        if nfill == 0:
                            l1p0_ps = l1_begin(0, x_n, 0)
                        l1_end(0, x_n, h1n, l1p0_ps, nfill)
                with pin(c, 2):
                    pss1 = hidden_kp0(1, h2)
                    h3 = hidden_kp1(1, h2, pss1)
                if not last:
                    with pin(c, 3):
                        l1_piece(1, x_n, h1n)

                # ---- out: delta^T = W3^T h3; out = delta^T + (zlast+b3)^T
                # out psums recycle the hidden-layer banks (ps2_0..2), so the
                # ps1 ring stays dedicated to the four L1' chains. Per-lt
                # kp0/kp1 pairs (instead of all-kp0-then-all-kp1) let each
                # Vector TT start as soon as its own pair stops, so the TT
                # drain overlaps the remaining out MMs.
                with pin(c, 4):
                    pso = [
                        pshpool.tile([128, NCH], f32, tag=f"ps2_{lt}", name=f"pso_{lt}")
                        for lt in range(LT)
                    ]
                    for lt in range(LT):
                        nc.tensor.matmul(
                            pso[lt][:],
                            w3_sb[:, 0, :, lt * 128 : (lt + 1) * 128],
                            h3[0][:],
                            start=True,
                            stop=False,
                            perf_mode=DR,
                        )
                        nc.tensor.matmul(
                            pso[lt][:],
                            w3_sb[:, 1, :, lt * 128 : (lt + 1) * 128],
                            h3[1][:],
                            start=False,
                            stop=True,
                            perf_mode=DR,
                        )
                        ot = opool.tile([128, NCH], bf16, tag=f"o{lt}", name=f"o{lt}")
                        nc.vector.tensor_tensor(ot[:], pso[lt][:], z_c[:, lt, :], add)
                        nc.sync.dma_start(
                            out=outT[
                                lt * 128 : (lt + 1) * 128, c * NCH : (c + 1) * NCH
                            ],
                            in_=ot[:],
                        )
                if not last:
                    with pin(c, 5):
                        l1_piece(2, x_n, h1n)
                    with pin(c, 6):
                        l1_piece(3, x_n, h1n, colsplit=True)
                    h1c, z_c = h1n, z_n
                    x_n = x_n2
                    x_n2 = x_f if c + 3 < nchunk else None
    nc.compile()
    return nc


def _f8():
    import ml_dtypes

    return ml_dtypes.float8_e4m3


def prep_core_inputs(
    z_hist, a_hist, W1, b1, Wh, bh, gamma, beta, rmean, rvar, W3, b3
):
    """Host-side shard prep: returns per-model input dicts (xT shared)."""
    f8 = _f8()
    batch = z_hist.shape[0]
    nchunk = batch // NCH
    x = np.concatenate(
        [z_hist.reshape(batch, -1), a_hist.reshape(batch, -1)], axis=1
    ).astype(np.float32)
    xpad = np.zeros((batch, DINP), np.float32)
    xpad[:, :DIN] = x
    xq = xpad.astype(f8)  # quantize once, then pure byte shuffles
    xT8 = np.ascontiguousarray(
        xq.reshape(nchunk, NCH, DINP // 128, 128).transpose(3, 0, 2, 1)
    ).reshape(128, nchunk, KP1, 2, NCH)

    z_last = z_hist[:, -1, :].astype(np.float32)  # [batch, L]

    rstd = 1.0 / np.sqrt(rvar.astype(np.float64) + EPS)  # [NHL, M, HID]
    s_aff = (gamma * rstd).astype(np.float32)
    c_aff = ((bh - rmean) * gamma * rstd + beta).astype(np.float32)

    in_maps = []
    for m in range(M):
        w1p = np.zeros((DINP, HID), np.float32)
        w1p[:DIN] = W1[m] * WS
        w1h = np.ascontiguousarray(
            w1p.astype(f8).reshape(DINP // 128, 128, HID).transpose(1, 0, 2)
        ).reshape(128, KP1, 2, HID)

        whh = np.ascontiguousarray(
            (Wh[:, m] * WS)
            .astype(np.float32)
            .astype(f8)
            .reshape(NHL, HT, 128, HT, 128)
            .transpose(2, 0, 1, 3, 4)
        ).reshape(128, NHL, HP, 2, HT, 128)

        w3h = np.ascontiguousarray(
            W3[m].astype(np.float32).astype(f8).reshape(HT, 128, L).transpose(1, 0, 2)
        ).reshape(128, HP, 2, L)

        vecs = np.zeros((128, NVEC), np.float32)
        vecs[:, COL_B1 : COL_B1 + HT] = b1[m].reshape(HT, 128).T
        for l in range(NHL):
            vecs[:, COL_S(l) : COL_S(l) + HT] = (s_aff[l, m] / WS).reshape(HT, 128).T
            vecs[:, COL_C(l) : COL_C(l) + HT] = c_aff[l, m].reshape(HT, 128).T
        vecs[:, COL_SL1] = 1.0 / WS

        zb = z_last + b3[m][None, :]  # fold b3 into the residual stream
        # bf16 residual + bf16 output: halves the z-in and out DMA streams
        # (the DMA system was the steady-state pacing limit at 2.5MB/period);
        # adds ~2e-3 rel-err against a 2e-2 gate
        import ml_dtypes

        zTm = np.ascontiguousarray(
            zb.reshape(nchunk, NCH, LT, 128).transpose(3, 0, 2, 1)
        ).astype(ml_dtypes.bfloat16)  # [128, nchunk, LT, NCH]
        in_maps.append(
            {"xT": xT8, "zT": zTm, "w1": w1h, "wh": whh, "w3": w3h, "vecs": vecs}
        )
    return in_maps


def _reset_device():
    """Clear any exec-unit wedge a previous (profiled) session left behind."""
    try:
        import ctypes

        import jax

        jax.devices()
        lib = ctypes.CDLL("/opt/axon/libaxon_pjrt.so")
        if hasattr(lib, "axon_reset"):
            lib.axon_reset.restype = ctypes.c_int64
            lib.axon_reset()
    except Exception:
        pass


def is_zero_bias(inputs):
    """True iff every additive term of the per-layer affines is zero, i.e.
    the activations reduce to relu(scale * psum)."""
    return not (
        inputs["b1"].any()
        or inputs["bh"].any()
        or inputs["beta"].any()
        or inputs["rmean"].any()
    )


def kernel(**inputs):
    inputs = {k: np.asarray(v) for k, v in inputs.items()}
    in_maps = prep_core_inputs(**inputs)
    nc = build_bass(B, zero_bias=is_zero_bias(inputs))

    from concourse import bass_utils

    _reset_device()
    res = bass_utils.run_bass_kernel_spmd(nc, in_maps, core_ids=list(range(M)))
    out = np.stack(
        [np.ascontiguousarray(res.results[m]["outT"].T) for m in range(M)]
    )  # [M, B, L]
    return out.astype(np.float32)



# revision 43
# speedup vs baseline: 1.0273x; 1.0025x over previous
"""Trainium2 Bass kernel for nn_EnsembleTransitionModel.

Sharding: model-parallel (expert-parallel). M=8 ensemble members across 8
NeuronCores; each core runs one full MLP over the whole batch. Inputs are
replicated, per-model weights are sharded.

All four matmul layers run in fp8 (e4m3) DoubleRow perf mode: each matmul
instruction contracts K=256 (two 128-k-tiles packed as pairs in the free
dim of both operands) at 2 MACs/cell/cycle — 2x bf16 throughput. fp32
accumulation in PSUM.

Precision plan (measured rel-err ~2.2e-3 vs the 2e-2 gate):
  - W1, Wh are scaled x64 before fp8 quantization (raw weights ~N(0,0.02)
    sit in e4m3's subnormal range); the 1/64 rides the existing per-feature
    affine applied by the scalar-engine Relu activation out of PSUM.
  - W3 stays unscaled (error is the same either way) so the output stage is
    a single fp32 vector add of the residual stream, exactly as in bf16.
  - The residual z_last (+ b3 folded) rides a separate fp32 path so the
    dominant output term stays full precision.
  - DIN=1925 is zero-padded to 2048 so the 5 a_hist rows fold into the main
    L1 matmul (no separate host-computed rank-5 term).

Layouts: activations feature-major (x^T: [features, batch]); x is packed
chunk-major in DRAM ([128, chunk, kpair, 2, 512]) so each chunk's DMA is
one fully-contiguous 8KB-per-partition transfer.
"""

import os
import sys

import numpy as np

for _p in ("/opt/trn_rl_repo", "/root/.axon_site/_ro/trn_rl_repo"):
    if os.path.isdir(_p) and _p not in sys.path:
        sys.path.insert(0, _p)

M = 8
B = 16384
HIST = 5
L = 384
A = 1
HID = 512
NHL = 2
DIN = L * HIST + A * HIST  # 1925
EPS = 1e-5

WS = 64.0  # fp8 weight pre-scale for W1/Wh (compensated in the affine)

NCH = 512  # batch columns per chunk (= max fp32 moving dim = 1 PSUM bank)
DINP = 2048  # DIN zero-padded to 16 k-tiles
KP1 = DINP // 256  # 8 L1 k-pairs (DoubleRow consumes 2 k-tiles per matmul)
HT = HID // 128  # 4 hidden feature tiles
HP = HT // 2  # 2 hidden k-pairs
LT = L // 128  # 3 output feature tiles
ZROW0 = (HIST - 1) * L  # 1536: first row of z_last within x^T

# vecs columns: [b1 (4) | s0 (4) | c0 (4) | s1 (4) | c1 (4) | sL1 (1)]
COL_B1 = 0
COL_S = lambda l: 4 + 8 * l
COL_C = lambda l: 8 + 8 * l
COL_SL1 = 4 + 8 * NHL
NVEC = COL_SL1 + 1


def build_bass(batch=B, zero_bias=True):
    """zero_bias=True (true for this model instance: b1/bh/beta/rmean all
    zero) routes half the activations to the Vector engine as a one-pass
    relu(scale*psum) tensor_scalar, halving the end-of-layer activation
    barrier the PE waits on. With nonzero biases everything stays on the
    Scalar engine's general affine activation path."""
    import concourse.bacc as bacc
    import concourse.tile as tile
    from concourse import mybir

    f32 = mybir.dt.float32
    bf16 = mybir.dt.bfloat16
    f8 = mybir.dt.float8e4
    DR = mybir.MatmulPerfMode.DoubleRow
    Relu = mybir.ActivationFunctionType.Relu
    add = mybir.AluOpType.add
    mult = mybir.AluOpType.mult
    maxop = mybir.AluOpType.max

    nchunk = batch // NCH
    assert nchunk * NCH == batch

    nc = bacc.Bacc("TRN2", target_bir_lowering=False)
    xT = nc.declare_dram_parameter("xT", [128, nchunk, KP1, 2, NCH], f8, isOutput=False)
    zT = nc.declare_dram_parameter("zT", [128, nchunk, LT, NCH], bf16, isOutput=False)
    w1 = nc.declare_dram_parameter("w1", [128, KP1, 2, HID], f8, isOutput=False)
    wh = nc.declare_dram_parameter("wh", [128, NHL, HP, 2, HT, 128], f8, isOutput=False)
    w3 = nc.declare_dram_parameter("w3", [128, HP, 2, L], f8, isOutput=False)
    vecs = nc.declare_dram_parameter("vecs", [128, NVEC], f32, isOutput=False)
    outT = nc.declare_dram_parameter("outT", [L, batch], bf16, isOutput=True)

    with tile.TileContext(nc) as tc:
        with (
            tc.tile_pool(name="wt", bufs=1) as wpool,
            tc.tile_pool(name="x", bufs=4) as xpool,
            tc.tile_pool(name="z", bufs=2) as zpool,
            tc.tile_pool(name="h", bufs=2) as hpool,
            tc.tile_pool(name="o", bufs=3) as opool,
            tc.tile_pool(name="ps1", bufs=4, space="PSUM") as ps1pool,
            tc.tile_pool(name="psh", bufs=1, space="PSUM") as pshpool,
        ):
            # per-k-pair W1 tiles so the first matmul only waits on its own
            # 128KB slice, not the whole 1MB preload (DMAs are interleaved
            # with chunk-0 x slices, kp by kp)
            w1_sb = [
                wpool.tile([128, 2, HID], f8, tag=f"w1_{kp}", name=f"w1_{kp}")
                for kp in range(KP1)
            ]
            wh_sb = wpool.tile([128, NHL, HP, 2, HT, 128], f8, tag="wh")
            w3_sb = wpool.tile([128, HP, 2, L], f8, tag="w3")
            v_sb = wpool.tile([128, NVEC], f32, tag="vecs")

            # The PE never idles in steady state: the four L1 matmul chains
            # of chunk c+1 (1.7us of act-independent work each) are emitted
            # inside chunk c's hidden/out phases so every end-of-layer
            # activation barrier is covered by L1' work:
            #   [hl0 kp0|kp1] L1' ht0 [hl1 kp0|kp1] L1' ht1+ht2
            #   [out kp0|kp1] L1' ht3
            def act(out_sl, ps_sl, scol, bcol, eng, l1=False):
                # out = relu(scale*ps + bias); bias==0 on the fast V path
                if eng == "V" and zero_bias:
                    sc = 1.0 / WS if l1 else v_sb[:, scol : scol + 1]
                    nc.vector.tensor_scalar(out_sl, ps_sl, sc, 0.0, mult, maxop)
                else:
                    sc = COL_SL1 if l1 else scol
                    nc.scalar.activation(
                        out_sl,
                        ps_sl,
                        Relu,
                        bias=v_sb[:, bcol : bcol + 1],
                        scale=v_sb[:, sc : sc + 1],
                    )

            def l1_chain(ht, x_t, ps):
                for kp in range(KP1):
                    nc.tensor.matmul(
                        ps[:],
                        w1_sb[kp][:, :, ht * 128 : (ht + 1) * 128],
                        x_t[:, kp],
                        start=(kp == 0),
                        stop=(kp == KP1 - 1),
                        perf_mode=DR,
                    )

            # Chain-act engine assignment. ht2's act queues on V right after
            # the TT adds (both done ~1us before the boundary); ht3's two
            # half-acts split S (first half, overlaps chainB) / V (second
            # half, V idle by then, and DVE tensor_scalar at 256 cols is
            # ~80ns faster than the Scalar path) — the ht3B act is the
            # period-boundary gate for next chunk's hidden kp1 group.
            L1_ENG = ("V", "V", "V", ("S", "V"))

            def l1_piece(ht, x_t, h1n, colsplit=False):
                ps = ps1pool.tile([128, NCH], f32, tag="ps1", name=f"psl1_{ht}")
                dst = h1n[ht // 2]
                eng = L1_ENG[ht] if zero_bias else "S"
                if colsplit:
                    # ht3's act gates next chunk's hidden kp1 group at the
                    # period boundary. Run the chain as two 256-col
                    # half-chains into SEPARATE psum banks (same-bank PE-W +
                    # ACT-R would serialize): the first half's act overlaps
                    # the second half's matmuls, so h1'[1] is ready ~250ns
                    # after the chain instead of ~1us.
                    half = NCH // 2
                    for h_ in range(2):
                        sl = slice(h_ * half, (h_ + 1) * half)
                        psh_ = ps if h_ == 0 else ps1pool.tile(
                            [128, NCH], f32, tag="ps1", name=f"psl1_{ht}b"
                        )
                        for kp in range(KP1):
                            nc.tensor.matmul(
                                psh_[:, :half],
                                w1_sb[kp][:, :, ht * 128 : (ht + 1) * 128],
                                x_t[:, kp, :, sl],
                                start=(kp == 0),
                                stop=(kp == KP1 - 1),
                                perf_mode=DR,
                            )
                        act(dst[:, ht % 2, sl], psh_[:, :half], None,
                            COL_B1 + ht, eng[h_] if zero_bias else "S",
                            l1=True)
                else:
                    l1_chain(ht, x_t, ps)
                    act(dst[:, ht % 2, :], ps[:], None, COL_B1 + ht, eng, l1=True)

            def l1_begin(ht, x_t, n0):
                # open an L1' chain with its first n0 matmuls — used to slot
                # act-independent PE work between hl0's kp0 and kp1 groups
                # so the period-boundary act gate has two extra slots of slack
                ps = ps1pool.tile([128, NCH], f32, tag="ps1", name=f"psl1_{ht}")
                for kp in range(n0):
                    nc.tensor.matmul(
                        ps[:],
                        w1_sb[kp][:, :, ht * 128 : (ht + 1) * 128],
                        x_t[:, kp],
                        start=(kp == 0),
                        stop=False,
                        perf_mode=DR,
                    )
                return ps

            def l1_end(ht, x_t, h1n, ps, n0):
                for kp in range(n0, KP1):
                    nc.tensor.matmul(
                        ps[:],
                        w1_sb[kp][:, :, ht * 128 : (ht + 1) * 128],
                        x_t[:, kp],
                        start=(kp == 0),
                        stop=(kp == KP1 - 1),
                        perf_mode=DR,
                    )
                act(h1n[ht // 2][:, ht % 2, :], ps[:], None, COL_B1 + ht,
                    L1_ENG[ht] if zero_bias else "S", l1=True)

            def hidden_kp0(l, hin):
                pss = [
                    pshpool.tile([128, NCH], f32, tag=f"ps2_{mt}", name=f"ps2_{mt}")
                    for mt in range(HT)
                ]
                for mt in range(HT):
                    nc.tensor.matmul(
                        pss[mt][:],
                        wh_sb[:, l, 0, :, mt, :],
                        hin[0][:],
                        start=True,
                        stop=False,
                        perf_mode=DR,
                    )
                return pss

            def hidden_kp1(l, hin, pss):
                hout = [
                    hpool.tile(
                        [128, 2, NCH], f8, tag=f"h{l + 2}_{p}", name=f"h{l + 2}_{p}"
                    )
                    for p in range(HP)
                ]
                for mt in range(HT):
                    nc.tensor.matmul(
                        pss[mt][:],
                        wh_sb[:, l, 1, :, mt, :],
                        hin[1][:],
                        start=False,
                        stop=True,
                        perf_mode=DR,
                    )
                    dst = hout[mt // 2]
                    # full-tile acts, alternating engines: each ~760-690ns
                    # and mt0/mt1 (the halves of hout[0]) land on different
                    # engines in parallel, so hout[0] is ready ~1.0us after
                    # its stop-MM — and it's half the instruction count of
                    # the S/V half-split (the ~360ns fixed cost per act
                    # instruction dominated the split's latency win)
                    eng = ("S" if mt % 2 == 0 else "V") if zero_bias else "S"
                    act(
                        dst[:, mt % 2, :],
                        pss[mt][:],
                        COL_S(l) + mt,
                        COL_C(l) + mt,
                        eng,
                    )
                return hout

            # ---- prologue: weights + x0 (sliced) + x1/x2, chunk-0 L1 ----
            # PE warm-up: the HAM clock gate holds the PE at 1.2GHz until it
            # has been busy ~3.4us. The first real matmul can't start before
            # the prologue DMAs land (~11us), so burn the wait on junk
            # matmuls over a memset tile — the real stream then opens at
            # 2.4GHz instead of paying ~1.7us of cold-clock penalty.
            warm_sb = wpool.tile([128, 2, NCH], f8, tag="warm")
            nc.vector.memset(warm_sb[:], 0.0)
            warm_ps = ps1pool.tile([128, NCH], f32, tag="ps1", name="warm_ps")
            NWARM = 8  # 8 cold MMs = ~3.4us = exactly one HAM window; ends
            # right as the first w1/x0 slices land (~11us)
            for i in range(NWARM):
                nc.tensor.matmul(
                    warm_ps[:],
                    warm_sb[:, :, :128],
                    warm_sb[:],
                    start=(i == 0),
                    stop=(i == NWARM - 1),
                    perf_mode=DR,
                )
            # split trigger issue across two queues: each DMA_DIRECT2D costs
            # ~600ns on its issuing queue, and 19 serial prologue triggers on
            # Sync alone (~12us) were pacing the whole warm-up region.
            # GpSimd is otherwise idle and can dispatch DMAs on trn2.
            x_c = xpool.tile([128, KP1, 2, NCH], f8, tag="x", name="x_0")
            x_n = None
            x_n2 = None
            if nchunk > 1:
                x_n = xpool.tile([128, KP1, 2, NCH], f8, tag="x", name="x_1")
            if nchunk > 2:
                x_n2 = xpool.tile([128, KP1, 2, NCH], f8, tag="x", name="x_2")
            for kp in range(KP1):
                nc.gpsimd.dma_start(out=w1_sb[kp][:], in_=w1[:, kp])
                nc.sync.dma_start(out=x_c[:, kp], in_=xT[:, 0, kp])
            if x_n is not None:
                nc.sync.dma_start(out=x_n[:], in_=xT[:, 1])
            if x_n2 is not None:
                nc.sync.dma_start(out=x_n2[:], in_=xT[:, 2])
            z_c = zpool.tile([128, LT, NCH], bf16, tag="z", name="z_0")
            nc.gpsimd.dma_start(out=z_c[:], in_=zT[:, 0])
            nc.gpsimd.dma_start(out=wh_sb[:], in_=wh[:])
            nc.gpsimd.dma_start(out=w3_sb[:], in_=w3[:])
            nc.gpsimd.dma_start(out=v_sb[:], in_=vecs[:])
            h1c = [
                hpool.tile([128, 2, NCH], f8, tag=f"h1_{p}", name=f"h1_{p}")
                for p in range(HP)
            ]
            # chunk-0 L1 kp-OUTER with all four accumulations open at once:
            # each kp step needs only its own 256KB (w1_kp + x0_kp) slice,
            # so the PE starts ~0.7us after the first DMA lands and stays
            # ~DMA-paced instead of waiting for the full 2MB preload
            # (ht-outer chains stalled ~11us here).
            pr_ps = [
                ps1pool.tile([128, NCH], f32, tag="ps1", name=f"ps0_{ht}")
                for ht in range(HT)
            ]
            for kp in range(KP1):
                for ht in range(HT):
                    nc.tensor.matmul(
                        pr_ps[ht][:],
                        w1_sb[kp][:, :, ht * 128 : (ht + 1) * 128],
                        x_c[:, kp],
                        start=(kp == 0),
                        stop=(kp == KP1 - 1),
                        perf_mode=DR,
                    )
            for ht in range(HT):
                act(
                    h1c[ht // 2][:, ht % 2, :],
                    pr_ps[ht][:],
                    None,
                    COL_B1 + ht,
                    "S" if ht % 2 == 0 else "V",
                    l1=True,
                )

            # The Tile scheduler's cost model rates DR matmuls ~2x faster
            # than hardware (106.7ns vs 216ns measured), so its static order
            # leaves the period boundary uncovered: next chunk's hidden-l0
            # MMs get placed right after the out MMs and stall ~600ns each on
            # the Vector TT adds freeing the shared ps2 banks. tile_wait_until
            # pins each phase to a monotone sim-time grid so the static order
            # is exactly: hl0 | l1p0 | hl1 | l1p1 | out | l1p2 | l1p3 — the
            # two trailing L1' chains (3.5us of act-independent PE work)
            # cover the TT drain (~2.1us) at every period boundary.
            PIN_MS = 0.002  # 2us sim-grid per phase
            NPH = 7

            def pin(c, k):
                return tc.tile_wait_until((NPH * c + k) * PIN_MS)

            for c in range(nchunk):
                last = c == nchunk - 1
                with pin(c, 0):
                    # prefetch x three chunks ahead: the chain filler MMs
                    # between hl0's kp0/kp1 groups read x(c+1) right at the
                    # period boundary, so it must be fully resident by then
                    if c + 3 < nchunk:
                        x_f = xpool.tile([128, KP1, 2, NCH], f8, tag="x", name="x_f")
                        nc.gpsimd.dma_start(out=x_f[:], in_=xT[:, c + 3])
                    z_n = None
                    h1n = None
                    if not last:
                        z_n = zpool.tile([128, LT, NCH], bf16, tag="z", name="z_n")
                        nc.gpsimd.dma_start(out=z_n[:], in_=zT[:, c + 1])
                        h1n = [
                            hpool.tile(
                                [128, 2, NCH], f8, tag=f"h1_{p}", name=f"h1n_{p}"
                            )
                            for p in range(HP)
                        ]

                    pss0 = hidden_kp0(0, h1c)
                # two L1' chain MMs between hl0's kp0 and kp1 groups:
                # kp1 reads h1[1] whose last act lands ~0.8us after the
                # ht3 chain — these two slots turn a ~40ns race into a
                # ~0.5us margin. Chunk 0 skips the filler: its h1 comes
                # from the prologue (no late-act gate) and x1 is still in
                # flight at that point.
                nfill = 2 if c > 0 else 0
                if not last and nfill:
                    with pin(c, 0.4):
                        l1p0_ps = l1_begin(0, x_n, nfill)
                with pin(c, 0.7):
                    h2 = hidden_kp1(0, h1c, pss0)
                if not last:
                    with pin(c, 1):
                        if nfill == 0:
                            l1p0_ps = l1_begin(0, x_n, 0)
                        l1_end(0, x_n, h1n, l1p0_ps, nfill)
                with pin(c, 2):
                    pss1 = hidden_kp0(1, h2)
                    h3 = hidden_kp1(1, h2, pss1)
                if not last:
                    with pin(c, 3):
                        l1_piece(1, x_n, h1n)

                # ---- out: delta^T = W3^T h3; out = delta^T + (zlast+b3)^T
                # out psums recycle the hidden-layer banks (ps2_0..2), so the
                # ps1 ring stays dedicated to the four L1' chains. Per-lt
                # kp0/kp1 pairs (instead of all-kp0-then-all-kp1) let each
                # Vector TT start as soon as its own pair stops, so the TT
                # drain overlaps the remaining out MMs.
                with pin(c, 4):
                    pso = [
                        pshpool.tile([128, NCH], f32, tag=f"ps2_{lt}", name=f"pso_{lt}")
                        for lt in range(LT)
                    ]
                    for lt in range(LT):
                        nc.tensor.matmul(
                            pso[lt][:],
                            w3_sb[:, 0, :, lt * 128 : (lt + 1) * 128],
                            h3[0][:],
                            start=True,
                            stop=False,
                            perf_mode=DR,
                        )
                        nc.tensor.matmul(
                            pso[lt][:],
                            w3_sb[:, 1, :, lt * 128 : (lt + 1) * 128],
                            h3[1][:],
                            start=False,
                            stop=True,
                            perf_mode=DR,
                        )
                        ot = opool.tile([128, NCH], bf16, tag=f"o{lt}", name=f"o{lt}")
                        nc.vector.tensor_tensor(ot[:], pso[lt][:], z_c[:, lt, :], add)
                        nc.sync.dma_start(
                            out=outT[
                                lt * 128 : (lt + 1) * 128, c * NCH : (c + 1) * NCH
                            ],
                            in_=ot[:],
                        )
                if not last:
                    with pin(c, 5):
                        l1_piece(2, x_n, h1n)
                    with pin(c, 6):
                        l1_piece(3, x_n, h1n, colsplit=True)
                    h1c, z_c = h1n, z_n
                    x_n = x_n2
                    x_n2 = x_f if c + 3 < nchunk else None
    nc.compile()
    return nc


def _f8():
    import ml_dtypes

    return ml_dtypes.float8_e4m3


def prep_core_inputs(
    z_hist, a_hist, W1, b1, Wh, bh, gamma, beta, rmean, rvar, W3, b3
):
    """Host-side shard prep: returns per-model input dicts (xT shared)."""
    f8 = _f8()
    batch = z_hist.shape[0]
    nchunk = batch // NCH
    x = np.concatenate(
        [z_hist.reshape(batch, -1), a_hist.reshape(batch, -1)], axis=1
    ).astype(np.float32)
    xpad = np.zeros((batch, DINP), np.float32)
    xpad[:, :DIN] = x
    xq = xpad.astype(f8)  # quantize once, then pure byte shuffles
    xT8 = np.ascontiguousarray(
        xq.reshape(nchunk, NCH, DINP // 128, 128).transpose(3, 0, 2, 1)
    ).reshape(128, nchunk, KP1, 2, NCH)

    z_last = z_hist[:, -1, :].astype(np.float32)  # [batch, L]

    rstd = 1.0 / np.sqrt(rvar.astype(np.float64) + EPS)  # [NHL, M, HID]
    s_aff = (gamma * rstd).astype(np.float32)
    c_aff = ((bh - rmean) * gamma * rstd + beta).astype(np.float32)

    in_maps = []
    for m in range(M):
        w1p = np.zeros((DINP, HID), np.float32)
        w1p[:DIN] = W1[m] * WS
        w1h = np.ascontiguousarray(
            w1p.astype(f8).reshape(DINP // 128, 128, HID).transpose(1, 0, 2)
        ).reshape(128, KP1, 2, HID)

        whh = np.ascontiguousarray(
            (Wh[:, m] * WS)
            .astype(np.float32)
            .astype(f8)
            .reshape(NHL, HT, 128, HT, 128)
            .transpose(2, 0, 1, 3, 4)
        ).reshape(128, NHL, HP, 2, HT, 128)

        w3h = np.ascontiguousarray(
            W3[m].astype(np.float32).astype(f8).reshape(HT, 128, L).transpose(1, 0, 2)
        ).reshape(128, HP, 2, L)

        vecs = np.zeros((128, NVEC), np.float32)
        vecs[:, COL_B1 : COL_B1 + HT] = b1[m].reshape(HT, 128).T
        for l in range(NHL):
            vecs[:, COL_S(l) : COL_S(l) + HT] = (s_aff[l, m] / WS).reshape(HT, 128).T
            vecs[:, COL_C(l) : COL_C(l) + HT] = c_aff[l, m].reshape(HT, 128).T
        vecs[:, COL_SL1] = 1.0 / WS

        zb = z_last + b3[m][None, :]  # fold b3 into the residual stream
        # bf16 residual + bf16 output: halves the z-in and out DMA streams
        # (the DMA system was the steady-state pacing limit at 2.5MB/period);
        # adds ~2e-3 rel-err against a 2e-2 gate
        import ml_dtypes

        zTm = np.ascontiguousarray(
            zb.reshape(nchunk, NCH, LT, 128).transpose(3, 0, 2, 1)
        ).astype(ml_dtypes.bfloat16)  # [128, nchunk, LT, NCH]
        in_maps.append(
            {"xT": xT8, "zT": zTm, "w1": w1h, "wh": whh, "w3": w3h, "vecs": vecs}
        )
    return in_maps


def _reset_device():
    """Clear any exec-unit wedge a previous (profiled) session left behind."""
    try:
        import ctypes

        import jax

        jax.devices()
        lib = ctypes.CDLL("/opt/axon/libaxon_pjrt.so")
        if hasattr(lib, "axon_reset"):
            lib.axon_reset.restype = ctypes.c_int64
            lib.axon_reset()
    except Exception:
        pass


def is_zero_bias(inputs):
    """True iff every additive term of the per-layer affines is zero, i.e.
    the activations reduce to relu(scale * psum)."""
    return not (
        inputs["b1"].any()
        or inputs["bh"].any()
        or inputs["beta"].any()
        or inputs["rmean"].any()
    )


def kernel(**inputs):
    inputs = {k: np.asarray(v) for k, v in inputs.items()}
    in_maps = prep_core_inputs(**inputs)
    nc = build_bass(B, zero_bias=is_zero_bias(inputs))

    from concourse import bass_utils

    _reset_device()
    res = bass_utils.run_bass_kernel_spmd(nc, in_maps, core_ids=list(range(M)))
    out = np.stack(
        [np.ascontiguousarray(res.results[m]["outT"].T) for m in range(M)]
    )  # [M, B, L]
    return out.astype(np.float32)



# revision 44
# speedup vs baseline: 1.0281x; 1.0007x over previous
"""Trainium2 Bass kernel for nn_EnsembleTransitionModel.

Sharding: model-parallel (expert-parallel). M=8 ensemble members across 8
NeuronCores; each core runs one full MLP over the whole batch. Inputs are
replicated, per-model weights are sharded.

All four matmul layers run in fp8 (e4m3) DoubleRow perf mode: each matmul
instruction contracts K=256 (two 128-k-tiles packed as pairs in the free
dim of both operands) at 2 MACs/cell/cycle — 2x bf16 throughput. fp32
accumulation in PSUM.

Precision plan (measured rel-err ~3.2e-3 vs the 2e-2 gate):
  - W1, Wh are scaled x64 before fp8 quantization (raw weights ~N(0,0.02)
    sit in e4m3's subnormal range); the 1/64 rides the existing per-feature
    affine applied by the Relu activation out of PSUM.
  - W3 stays unscaled (error is the same either way) so the output stage is
    a single vector add of the residual stream.
  - The residual z_last (+ b3 folded) and the output ride a bf16 path
    (adds ~1e-3 rel-err; halves the z-in and out DMA streams, which were
    the steady-state DMA pacing limit at 2.5MB/chunk).
  - DIN=1925 is zero-padded to 2048 so the 5 a_hist rows fold into the main
    L1 matmul (no separate host-computed rank-5 term).

Layouts: activations feature-major (x^T: [features, batch]); x is packed
chunk-major in DRAM ([128, chunk, kpair, 2, 512]) so each chunk's DMA is
one fully-contiguous 8KB-per-partition transfer.

Scheduling (433us -> ~401us): the Tile scheduler's cost model rates fp8-DR
matmuls ~2x faster than hardware, so its static order left the PE idle at
every period boundary. tile_wait_until pins give each chunk a 7-phase
static order [hl0-kp0 | fill | hl0-kp1+l1'ht0 | hl1 | l1'ht1 | out+TT |
l1'ht2 | l1'ht3(col-split)] in which every activation/TT latency window is
covered by act-independent L1' chain matmuls; x is prefetched 3 chunks
ahead; prologue DMA triggers are split across the Sync and GpSimd queues;
8 junk matmuls on a memset tile warm the PE HAM clock-gate during the
initial DMA wait.
"""

import os
import sys

import numpy as np

for _p in ("/opt/trn_rl_repo", "/root/.axon_site/_ro/trn_rl_repo"):
    if os.path.isdir(_p) and _p not in sys.path:
        sys.path.insert(0, _p)

M = 8
B = 16384
HIST = 5
L = 384
A = 1
HID = 512
NHL = 2
DIN = L * HIST + A * HIST  # 1925
EPS = 1e-5

WS = 64.0  # fp8 weight pre-scale for W1/Wh (compensated in the affine)

NCH = 512  # batch columns per chunk (= max fp32 moving dim = 1 PSUM bank)
DINP = 2048  # DIN zero-padded to 16 k-tiles
KP1 = DINP // 256  # 8 L1 k-pairs (DoubleRow consumes 2 k-tiles per matmul)
HT = HID // 128  # 4 hidden feature tiles
HP = HT // 2  # 2 hidden k-pairs
LT = L // 128  # 3 output feature tiles
ZROW0 = (HIST - 1) * L  # 1536: first row of z_last within x^T

# vecs columns: [b1 (4) | s0 (4) | c0 (4) | s1 (4) | c1 (4) | sL1 (1)]
COL_B1 = 0
COL_S = lambda l: 4 + 8 * l
COL_C = lambda l: 8 + 8 * l
COL_SL1 = 4 + 8 * NHL
NVEC = COL_SL1 + 1


def build_bass(batch=B, zero_bias=True):
    """zero_bias=True (true for this model instance: b1/bh/beta/rmean all
    zero) routes half the activations to the Vector engine as a one-pass
    relu(scale*psum) tensor_scalar, halving the end-of-layer activation
    barrier the PE waits on. With nonzero biases everything stays on the
    Scalar engine's general affine activation path."""
    import concourse.bacc as bacc
    import concourse.tile as tile
    from concourse import mybir

    f32 = mybir.dt.float32
    bf16 = mybir.dt.bfloat16
    f8 = mybir.dt.float8e4
    DR = mybir.MatmulPerfMode.DoubleRow
    Relu = mybir.ActivationFunctionType.Relu
    add = mybir.AluOpType.add
    mult = mybir.AluOpType.mult
    maxop = mybir.AluOpType.max

    nchunk = batch // NCH
    assert nchunk * NCH == batch

    nc = bacc.Bacc("TRN2", target_bir_lowering=False)
    xT = nc.declare_dram_parameter("xT", [128, nchunk, KP1, 2, NCH], f8, isOutput=False)
    zT = nc.declare_dram_parameter("zT", [128, nchunk, LT, NCH], bf16, isOutput=False)
    w1 = nc.declare_dram_parameter("w1", [128, KP1, 2, HID], f8, isOutput=False)
    wh = nc.declare_dram_parameter("wh", [128, NHL, HP, 2, HT, 128], f8, isOutput=False)
    w3 = nc.declare_dram_parameter("w3", [128, HP, 2, L], f8, isOutput=False)
    vecs = nc.declare_dram_parameter("vecs", [128, NVEC], f32, isOutput=False)
    outT = nc.declare_dram_parameter("outT", [L, batch], bf16, isOutput=True)

    with tile.TileContext(nc) as tc:
        with (
            tc.tile_pool(name="wt", bufs=1) as wpool,
            tc.tile_pool(name="x", bufs=4) as xpool,
            tc.tile_pool(name="z", bufs=2) as zpool,
            tc.tile_pool(name="h", bufs=2) as hpool,
            tc.tile_pool(name="o", bufs=3) as opool,
            tc.tile_pool(name="ps1", bufs=4, space="PSUM") as ps1pool,
            tc.tile_pool(name="psh", bufs=1, space="PSUM") as pshpool,
        ):
            # per-k-pair W1 tiles so the first matmul only waits on its own
            # 128KB slice, not the whole 1MB preload (DMAs are interleaved
            # with chunk-0 x slices, kp by kp)
            w1_sb = [
                wpool.tile([128, 2, HID], f8, tag=f"w1_{kp}", name=f"w1_{kp}")
                for kp in range(KP1)
            ]
            wh_sb = wpool.tile([128, NHL, HP, 2, HT, 128], f8, tag="wh")
            w3_sb = wpool.tile([128, HP, 2, L], f8, tag="w3")
            v_sb = wpool.tile([128, NVEC], f32, tag="vecs")

            # The PE never idles in steady state: the four L1 matmul chains
            # of chunk c+1 (1.7us of act-independent work each) are emitted
            # inside chunk c's hidden/out phases so every end-of-layer
            # activation barrier is covered by L1' work:
            #   [hl0 kp0|kp1] L1' ht0 [hl1 kp0|kp1] L1' ht1+ht2
            #   [out kp0|kp1] L1' ht3
            def act(out_sl, ps_sl, scol, bcol, eng, l1=False):
                # out = relu(scale*ps + bias); bias==0 on the fast V path
                if eng == "V" and zero_bias:
                    sc = 1.0 / WS if l1 else v_sb[:, scol : scol + 1]
                    nc.vector.tensor_scalar(out_sl, ps_sl, sc, 0.0, mult, maxop)
                else:
                    sc = COL_SL1 if l1 else scol
                    nc.scalar.activation(
                        out_sl,
                        ps_sl,
                        Relu,
                        bias=v_sb[:, bcol : bcol + 1],
                        scale=v_sb[:, sc : sc + 1],
                    )

            def l1_chain(ht, x_t, ps):
                for kp in range(KP1):
                    nc.tensor.matmul(
                        ps[:],
                        w1_sb[kp][:, :, ht * 128 : (ht + 1) * 128],
                        x_t[:, kp],
                        start=(kp == 0),
                        stop=(kp == KP1 - 1),
                        perf_mode=DR,
                    )

            # Chain-act engine assignment. ht2's act queues on V right after
            # the TT adds (both done ~1us before the boundary); ht3's two
            # half-acts split S (first half, overlaps chainB) / V (second
            # half, V idle by then, and DVE tensor_scalar at 256 cols is
            # ~80ns faster than the Scalar path) — the ht3B act is the
            # period-boundary gate for next chunk's hidden kp1 group.
            L1_ENG = ("V", "V", "V", ("S", "V"))

            def l1_piece(ht, x_t, h1n, colsplit=False):
                ps = ps1pool.tile([128, NCH], f32, tag="ps1", name=f"psl1_{ht}")
                dst = h1n[ht // 2]
                eng = L1_ENG[ht] if zero_bias else "S"
                if colsplit:
                    # ht3's act gates next chunk's hidden kp1 group at the
                    # period boundary. Run the chain as two 256-col
                    # half-chains into SEPARATE psum banks (same-bank PE-W +
                    # ACT-R would serialize): the first half's act overlaps
                    # the second half's matmuls, so h1'[1] is ready ~250ns
                    # after the chain instead of ~1us.
                    half = NCH // 2
                    for h_ in range(2):
                        sl = slice(h_ * half, (h_ + 1) * half)
                        psh_ = ps if h_ == 0 else ps1pool.tile(
                            [128, NCH], f32, tag="ps1", name=f"psl1_{ht}b"
                        )
                        for kp in range(KP1):
                            nc.tensor.matmul(
                                psh_[:, :half],
                                w1_sb[kp][:, :, ht * 128 : (ht + 1) * 128],
                                x_t[:, kp, :, sl],
                                start=(kp == 0),
                                stop=(kp == KP1 - 1),
                                perf_mode=DR,
                            )
                        act(dst[:, ht % 2, sl], psh_[:, :half], None,
                            COL_B1 + ht, eng[h_] if zero_bias else "S",
                            l1=True)
                else:
                    l1_chain(ht, x_t, ps)
                    act(dst[:, ht % 2, :], ps[:], None, COL_B1 + ht, eng, l1=True)

            def l1_begin(ht, x_t, n0):
                # open an L1' chain with its first n0 matmuls — used to slot
                # act-independent PE work between hl0's kp0 and kp1 groups
                # so the period-boundary act gate has two extra slots of slack
                ps = ps1pool.tile([128, NCH], f32, tag="ps1", name=f"psl1_{ht}")
                for kp in range(n0):
                    nc.tensor.matmul(
                        ps[:],
                        w1_sb[kp][:, :, ht * 128 : (ht + 1) * 128],
                        x_t[:, kp],
                        start=(kp == 0),
                        stop=False,
                        perf_mode=DR,
                    )
                return ps

            def l1_end(ht, x_t, h1n, ps, n0):
                for kp in range(n0, KP1):
                    nc.tensor.matmul(
                        ps[:],
                        w1_sb[kp][:, :, ht * 128 : (ht + 1) * 128],
                        x_t[:, kp],
                        start=(kp == 0),
                        stop=(kp == KP1 - 1),
                        perf_mode=DR,
                    )
                act(h1n[ht // 2][:, ht % 2, :], ps[:], None, COL_B1 + ht,
                    L1_ENG[ht] if zero_bias else "S", l1=True)

            def hidden_kp0(l, hin):
                pss = [
                    pshpool.tile([128, NCH], f32, tag=f"ps2_{mt}", name=f"ps2_{mt}")
                    for mt in range(HT)
                ]
                for mt in range(HT):
                    nc.tensor.matmul(
                        pss[mt][:],
                        wh_sb[:, l, 0, :, mt, :],
                        hin[0][:],
                        start=True,
                        stop=False,
                        perf_mode=DR,
                    )
                return pss

            def hidden_kp1(l, hin, pss):
                hout = [
                    hpool.tile(
                        [128, 2, NCH], f8, tag=f"h{l + 2}_{p}", name=f"h{l + 2}_{p}"
                    )
                    for p in range(HP)
                ]
                for mt in range(HT):
                    nc.tensor.matmul(
                        pss[mt][:],
                        wh_sb[:, l, 1, :, mt, :],
                        hin[1][:],
                        start=False,
                        stop=True,
                        perf_mode=DR,
                    )
                    dst = hout[mt // 2]
                    # full-tile acts, alternating engines: each ~760-690ns
                    # and mt0/mt1 (the halves of hout[0]) land on different
                    # engines in parallel, so hout[0] is ready ~1.0us after
                    # its stop-MM — and it's half the instruction count of
                    # the S/V half-split (the ~360ns fixed cost per act
                    # instruction dominated the split's latency win)
                    eng = ("S" if mt % 2 == 0 else "V") if zero_bias else "S"
                    act(
                        dst[:, mt % 2, :],
                        pss[mt][:],
                        COL_S(l) + mt,
                        COL_C(l) + mt,
                        eng,
                    )
                return hout

            # ---- prologue: weights + x0 (sliced) + x1/x2, chunk-0 L1 ----
            # PE warm-up: the HAM clock gate holds the PE at 1.2GHz until it
            # has been busy ~3.4us. The first real matmul can't start before
            # the prologue DMAs land (~11us), so burn the wait on junk
            # matmuls over a memset tile — the real stream then opens at
            # 2.4GHz instead of paying ~1.7us of cold-clock penalty.
            warm_sb = wpool.tile([128, 2, NCH], f8, tag="warm")
            nc.vector.memset(warm_sb[:], 0.0)
            warm_ps = ps1pool.tile([128, NCH], f32, tag="ps1", name="warm_ps")
            NWARM = 8  # 8 cold MMs = ~3.4us = exactly one HAM window; ends
            # right as the first w1/x0 slices land (~11us)
            for i in range(NWARM):
                nc.tensor.matmul(
                    warm_ps[:],
                    warm_sb[:, :, :128],
                    warm_sb[:],
                    start=(i == 0),
                    stop=(i == NWARM - 1),
                    perf_mode=DR,
                )
            # split trigger issue across two queues: each DMA_DIRECT2D costs
            # ~600ns on its issuing queue, and 19 serial prologue triggers on
            # Sync alone (~12us) were pacing the whole warm-up region.
            # GpSimd is otherwise idle and can dispatch DMAs on trn2.
            x_c = xpool.tile([128, KP1, 2, NCH], f8, tag="x", name="x_0")
            x_n = None
            x_n2 = None
            if nchunk > 1:
                x_n = xpool.tile([128, KP1, 2, NCH], f8, tag="x", name="x_1")
            if nchunk > 2:
                x_n2 = xpool.tile([128, KP1, 2, NCH], f8, tag="x", name="x_2")
            for kp in range(KP1):
                nc.gpsimd.dma_start(out=w1_sb[kp][:], in_=w1[:, kp])
                nc.sync.dma_start(out=x_c[:, kp], in_=xT[:, 0, kp])
            if x_n is not None:
                nc.sync.dma_start(out=x_n[:], in_=xT[:, 1])
            if x_n2 is not None:
                nc.sync.dma_start(out=x_n2[:], in_=xT[:, 2])
            z_c = zpool.tile([128, LT, NCH], bf16, tag="z", name="z_0")
            nc.gpsimd.dma_start(out=z_c[:], in_=zT[:, 0])
            nc.gpsimd.dma_start(out=wh_sb[:], in_=wh[:])
            nc.gpsimd.dma_start(out=w3_sb[:], in_=w3[:])
            nc.gpsimd.dma_start(out=v_sb[:], in_=vecs[:])
            h1c = [
                hpool.tile([128, 2, NCH], f8, tag=f"h1_{p}", name=f"h1_{p}")
                for p in range(HP)
            ]
            # chunk-0 L1 kp-OUTER with all four accumulations open at once:
            # each kp step needs only its own 256KB (w1_kp + x0_kp) slice,
            # so the PE starts ~0.7us after the first DMA lands and stays
            # ~DMA-paced instead of waiting for the full 2MB preload
            # (ht-outer chains stalled ~11us here).
            pr_ps = [
                ps1pool.tile([128, NCH], f32, tag="ps1", name=f"ps0_{ht}")
                for ht in range(HT)
            ]
            for kp in range(KP1):
                for ht in range(HT):
                    nc.tensor.matmul(
                        pr_ps[ht][:],
                        w1_sb[kp][:, :, ht * 128 : (ht + 1) * 128],
                        x_c[:, kp],
                        start=(kp == 0),
                        stop=(kp == KP1 - 1),
                        perf_mode=DR,
                    )
            for ht in range(HT):
                act(
                    h1c[ht // 2][:, ht % 2, :],
                    pr_ps[ht][:],
                    None,
                    COL_B1 + ht,
                    "S" if ht % 2 == 0 else "V",
                    l1=True,
                )

            # The Tile scheduler's cost model rates DR matmuls ~2x faster
            # than hardware (106.7ns vs 216ns measured), so its static order
            # leaves the period boundary uncovered: next chunk's hidden-l0
            # MMs get placed right after the out MMs and stall ~600ns each on
            # the Vector TT adds freeing the shared ps2 banks. tile_wait_until
            # pins each phase to a monotone sim-time grid so the static order
            # is exactly: hl0 | l1p0 | hl1 | l1p1 | out | l1p2 | l1p3 — the
            # two trailing L1' chains (3.5us of act-independent PE work)
            # cover the TT drain (~2.1us) at every period boundary.
            PIN_MS = 0.002  # 2us sim-grid per phase
            NPH = 7

            def pin(c, k):
                return tc.tile_wait_until((NPH * c + k) * PIN_MS)

            for c in range(nchunk):
                last = c == nchunk - 1
                with pin(c, 0):
                    # prefetch x three chunks ahead: the chain filler MMs
                    # between hl0's kp0/kp1 groups read x(c+1) right at the
                    # period boundary, so it must be fully resident by then
                    if c + 3 < nchunk:
                        x_f = xpool.tile([128, KP1, 2, NCH], f8, tag="x", name="x_f")
                        nc.gpsimd.dma_start(out=x_f[:], in_=xT[:, c + 3])
                    z_n = None
                    h1n = None
                    if not last:
                        z_n = zpool.tile([128, LT, NCH], bf16, tag="z", name="z_n")
                        nc.gpsimd.dma_start(out=z_n[:], in_=zT[:, c + 1])
                        h1n = [
                            hpool.tile(
                                [128, 2, NCH], f8, tag=f"h1_{p}", name=f"h1n_{p}"
                            )
                            for p in range(HP)
                        ]

                    pss0 = hidden_kp0(0, h1c)
                # two L1' chain MMs between hl0's kp0 and kp1 groups:
                # kp1 reads h1[1] whose last act lands ~0.8us after the
                # ht3 chain — these two slots turn a ~40ns race into a
                # ~0.5us margin. Chunk 0 skips the filler: its h1 comes
                # from the prologue (no late-act gate) and x1 is still in
                # flight at that point.
                nfill = 2 if c > 0 else 0
                if not last and nfill:
                    with pin(c, 0.4):
                        l1p0_ps = l1_begin(0, x_n, nfill)
                with pin(c, 0.7):
                    h2 = hidden_kp1(0, h1c, pss0)
                if not last:
                    with pin(c, 1):
                        if nfill == 0:
                            l1p0_ps = l1_begin(0, x_n, 0)
                        l1_end(0, x_n, h1n, l1p0_ps, nfill)
                with pin(c, 2):
                    pss1 = hidden_kp0(1, h2)
                    h3 = hidden_kp1(1, h2, pss1)
                if not last:
                    with pin(c, 3):
                        l1_piece(1, x_n, h1n)

                # ---- out: delta^T = W3^T h3; out = delta^T + (zlast+b3)^T
                # out psums recycle the hidden-layer banks (ps2_0..2), so the
                # ps1 ring stays dedicated to the four L1' chains. Per-lt
                # kp0/kp1 pairs (instead of all-kp0-then-all-kp1) let each
                # Vector TT start as soon as its own pair stops, so the TT
                # drain overlaps the remaining out MMs.
                with pin(c, 4):
                    pso = [
                        pshpool.tile([128, NCH], f32, tag=f"ps2_{lt}", name=f"pso_{lt}")
                        for lt in range(LT)
                    ]
                    for lt in range(LT):
                        nc.tensor.matmul(
                            pso[lt][:],
                            w3_sb[:, 0, :, lt * 128 : (lt + 1) * 128],
                            h3[0][:],
                            start=True,
                            stop=False,
                            perf_mode=DR,
                        )
                        nc.tensor.matmul(
                            pso[lt][:],
                            w3_sb[:, 1, :, lt * 128 : (lt + 1) * 128],
                            h3[1][:],
                            start=False,
                            stop=True,
                            perf_mode=DR,
                        )
                        ot = opool.tile([128, NCH], bf16, tag=f"o{lt}", name=f"o{lt}")
                        nc.vector.tensor_tensor(ot[:], pso[lt][:], z_c[:, lt, :], add)
                        nc.sync.dma_start(
                            out=outT[
                                lt * 128 : (lt + 1) * 128, c * NCH : (c + 1) * NCH
                            ],
                            in_=ot[:],
                        )
                if not last:
                    with pin(c, 5):
                        l1_piece(2, x_n, h1n)
                    with pin(c, 6):
                        l1_piece(3, x_n, h1n, colsplit=True)
                    h1c, z_c = h1n, z_n
                    x_n = x_n2
                    x_n2 = x_f if c + 3 < nchunk else None
    nc.compile()
    return nc


def _f8():
    import ml_dtypes

    return ml_dtypes.float8_e4m3


def prep_core_inputs(
    z_hist, a_hist, W1, b1, Wh, bh, gamma, beta, rmean, rvar, W3, b3
):
    """Host-side shard prep: returns per-model input dicts (xT shared)."""
    f8 = _f8()
    batch = z_hist.shape[0]
    nchunk = batch // NCH
    x = np.concatenate(
        [z_hist.reshape(batch, -1), a_hist.reshape(batch, -1)], axis=1
    ).astype(np.float32)
    xpad = np.zeros((batch, DINP), np.float32)
    xpad[:, :DIN] = x
    xq = xpad.astype(f8)  # quantize once, then pure byte shuffles
    xT8 = np.ascontiguousarray(
        xq.reshape(nchunk, NCH, DINP // 128, 128).transpose(3, 0, 2, 1)
    ).reshape(128, nchunk, KP1, 2, NCH)

    z_last = z_hist[:, -1, :].astype(np.float32)  # [batch, L]

    rstd = 1.0 / np.sqrt(rvar.astype(np.float64) + EPS)  # [NHL, M, HID]
    s_aff = (gamma * rstd).astype(np.float32)
    c_aff = ((bh - rmean) * gamma * rstd + beta).astype(np.float32)

    in_maps = []
    for m in range(M):
        w1p = np.zeros((DINP, HID), np.float32)
        w1p[:DIN] = W1[m] * WS
        w1h = np.ascontiguousarray(
            w1p.astype(f8).reshape(DINP // 128, 128, HID).transpose(1, 0, 2)
        ).reshape(128, KP1, 2, HID)

        whh = np.ascontiguousarray(
            (Wh[:, m] * WS)
            .astype(np.float32)
            .astype(f8)
            .reshape(NHL, HT, 128, HT, 128)
            .transpose(2, 0, 1, 3, 4)
        ).reshape(128, NHL, HP, 2, HT, 128)

        w3h = np.ascontiguousarray(
            W3[m].astype(np.float32).astype(f8).reshape(HT, 128, L).transpose(1, 0, 2)
        ).reshape(128, HP, 2, L)

        vecs = np.zeros((128, NVEC), np.float32)
        vecs[:, COL_B1 : COL_B1 + HT] = b1[m].reshape(HT, 128).T
        for l in range(NHL):
            vecs[:, COL_S(l) : COL_S(l) + HT] = (s_aff[l, m] / WS).reshape(HT, 128).T
            vecs[:, COL_C(l) : COL_C(l) + HT] = c_aff[l, m].reshape(HT, 128).T
        vecs[:, COL_SL1] = 1.0 / WS

        zb = z_last + b3[m][None, :]  # fold b3 into the residual stream
        # bf16 residual + bf16 output: halves the z-in and out DMA streams
        # (the DMA system was the steady-state pacing limit at 2.5MB/period);
        # adds ~2e-3 rel-err against a 2e-2 gate
        import ml_dtypes

        zTm = np.ascontiguousarray(
            zb.reshape(nchunk, NCH, LT, 128).transpose(3, 0, 2, 1)
        ).astype(ml_dtypes.bfloat16)  # [128, nchunk, LT, NCH]
        in_maps.append(
            {"xT": xT8, "zT": zTm, "w1": w1h, "wh": whh, "w3": w3h, "vecs": vecs}
        )
    return in_maps


def _reset_device():
    """Clear any exec-unit wedge a previous (profiled) session left behind."""
    try:
        import ctypes

        import jax

        jax.devices()
        lib = ctypes.CDLL("/opt/axon/libaxon_pjrt.so")
        if hasattr(lib, "axon_reset"):
            lib.axon_reset.restype = ctypes.c_int64
            lib.axon_reset()
    except Exception:
        pass


def is_zero_bias(inputs):
    """True iff every additive term of the per-layer affines is zero, i.e.
    the activations reduce to relu(scale * psum)."""
    return not (
        inputs["b1"].any()
        or inputs["bh"].any()
        or inputs["beta"].any()
        or inputs["rmean"].any()
    )


def kernel(**inputs):
    inputs = {k: np.asarray(v) for k, v in inputs.items()}
    in_maps = prep_core_inputs(**inputs)
    nc = build_bass(B, zero_bias=is_zero_bias(inputs))

    from concourse import bass_utils

    _reset_device()
    res = bass_utils.run_bass_kernel_spmd(nc, in_maps, core_ids=list(range(M)))
    out = np.stack(
        [np.ascontiguousarray(res.results[m]["outT"].T) for m in range(M)]
    )  # [M, B, L]
    return out.astype(np.float32)

